# revision 17
# baseline (speedup 1.0000x reference)
"""Graphformer encoder layer on 8 trn2 NeuronCores.

Sharding: sequence-parallel over queries. Every core receives the full
input (needed for K/V over all 4096 tokens) plus its own 512-row query
block, and produces 512 rows of the output; the host concatenates.
No collectives.

Layout strategy (per core):
  - LN1 in natural layout, PE-transpose to hT [256(2x128p), 4096].
  - Per head h: K^T [256, 4096] (d on partitions) and V natural
    [4096, 256] projected from hT (bf16).
  - Scores computed TRANSPOSED: S^T[keys, q] = K_h^T.T-chunks @ Q_h^T,
    so softmax sums over keys are partition-dim sums done with a
    ones-matmul on the PE, and probs feed the probs@V matmul without
    any transpose. Softmax normalization is delayed (applied to
    attnU = expS@V before the Wo matmul).
  - Wo output accumulated in PSUM across all 8 heads.
  - Residual/LN2/FFN on the core's own 512 rows.

LayerNorm gains/biases are folded into the (host-transposed) weights:
  Q = ln1norm(x) @ (Wq*g1).T + Wq@b1ln  (same K, V; FFN with ln2).
"""

import os
import sys
import numpy as np

try:
    import concourse.bass as bass
except Exception:  # pragma: no cover
    sys.path.insert(0, "/opt/trn_rl_repo")
    import concourse.bass as bass

import ml_dtypes
import concourse.tile as tile
from concourse import bacc, mybir
from concourse.bass_utils import run_bass_kernel_spmd
from concourse.masks import make_identity

F32 = mybir.dt.float32
F32R = mybir.dt.float32r
BF16 = mybir.dt.bfloat16
BF = ml_dtypes.bfloat16

N = 4096          # tokens
D = 256           # model dim
H = 8             # heads
DH = 256          # head dim  (2 chunks of 128)
HD = H * DH       # 2048
HID = 1024        # ffn hidden
NCORES = 8
NQ = N // NCORES  # 512 queries per core
SCALE = 1.0 / np.sqrt(np.float32(DH))
LN_EPS = 1e-5

KB = N // 128     # 32 key blocks of 128
QB = NQ // 128    # 4 query blocks of 128
TS = N // 512     # 8 token slabs of 512 (matmul moving max for fp32)


def _r(ap):
    """Bitcast fp32 -> float32r: full-rate (1 cyc/row) PE streaming."""
    return ap.bitcast(F32R)


def _ln_norm(nc, pool, xt, out):
    """out = (xt - mean(xt)) * rsqrt(var(xt)+eps), stats on DVE, sqrt on ACT."""
    stats = pool.tile([128, 6], F32, tag="ln_stats")
    nc.vector.bn_stats(out=stats, in_=xt)
    mv = pool.tile([128, 2], F32, tag="ln_mv")
    nc.vector.bn_aggr(out=mv, in_=stats)
    vpe = pool.tile([128, 1], F32, tag="ln_vpe")
    nc.vector.tensor_scalar_add(out=vpe, in0=mv[:, 1:2], scalar1=LN_EPS)
    std = pool.tile([128, 1], F32, tag="ln_std")
    nc.scalar.sqrt(std, vpe)
    rsig = pool.tile([128, 1], F32, tag="ln_rsig")
    nc.vector.reciprocal(out=rsig, in_=std)
    nc.vector.tensor_scalar(out=out, in0=xt, scalar1=mv[:, 0:1], scalar2=rsig,
                            op0=mybir.AluOpType.subtract,
                            op1=mybir.AluOpType.mult)


def _build():
    nc = bacc.Bacc("TRN2", target_bir_lowering=False, debug=False)

    dx = nc.dram_tensor("x", [N, D], F32, kind="ExternalInput").ap()
    dxq = nc.dram_tensor("xq", [NQ, D], F32, kind="ExternalInput").ap()
    dwq = nc.dram_tensor("wqt", [D, HD], BF16, kind="ExternalInput").ap()
    dwk = nc.dram_tensor("wkt", [D, HD], BF16, kind="ExternalInput").ap()
    dwv = nc.dram_tensor("wvt", [D, HD], BF16, kind="ExternalInput").ap()
    dbq = nc.dram_tensor("bqr", [128, HD // 128], F32, kind="ExternalInput").ap()
    dbk = nc.dram_tensor("bkr", [128, HD // 128], F32, kind="ExternalInput").ap()
    dbv = nc.dram_tensor("bvb", [128, HD], F32, kind="ExternalInput").ap()
    dwo = nc.dram_tensor("wot", [HD, D], BF16, kind="ExternalInput").ap()
    dbo = nc.dram_tensor("bor", [128, D // 128], F32, kind="ExternalInput").ap()
    dw1 = nc.dram_tensor("w1t", [D, HID], BF16, kind="ExternalInput").ap()
    db1 = nc.dram_tensor("b1r", [128, HID // 128], F32, kind="ExternalInput").ap()
    dw2 = nc.dram_tensor("w2t", [HID, D], BF16, kind="ExternalInput").ap()
    db2 = nc.dram_tensor("b2b", [128, D], F32, kind="ExternalInput").ap()
    dout = nc.dram_tensor("out", [NQ, D], F32, kind="ExternalOutput").ap()

    with tile.TileContext(nc) as tc:
        import contextlib
        with contextlib.ExitStack() as ctx:
            _body(ctx, tc, nc, dx, dxq, dwq, dwk, dwv, dbq, dbk, dbv,
                  dwo, dbo, dw1, db1, dw2, db2, dout)
    nc.compile()
    return nc


def _body(ctx, tc, nc, dx, dxq, dwq, dwk, dwv, dbq, dbk, dbv,
          dwo, dbo, dw1, db1, dw2, db2, dout):
    Act = mybir.ActivationFunctionType

    wp = ctx.enter_context(tc.tile_pool(name="weights", bufs=1))
    hp = ctx.enter_context(tc.tile_pool(name="persist", bufs=1))
    kvp = ctx.enter_context(tc.tile_pool(name="kv", bufs=1))
    work = ctx.enter_context(tc.tile_pool(name="work", bufs=3))
    sp = ctx.enter_context(tc.tile_pool(name="probs", bufs=3))
    ap2 = ctx.enter_context(tc.tile_pool(name="attn", bufs=2))
    accp = ctx.enter_context(tc.tile_pool(name="acc", bufs=1, space="PSUM"))
    psp = ctx.enter_context(tc.tile_pool(name="pscr", bufs=2, space="PSUM"))

    # ---- weights / constants ----
    wqt = wp.tile([128, 2, HD], BF16)
    nc.sync.dma_start(out=wqt, in_=dwq.rearrange("(b p) d -> p b d", p=128))
    wkt = wp.tile([128, 2, HD], BF16)
    nc.sync.dma_start(out=wkt, in_=dwk.rearrange("(b p) d -> p b d", p=128))
    wvt = wp.tile([128, 2, HD], BF16)
    nc.sync.dma_start(out=wvt, in_=dwv.rearrange("(b p) d -> p b d", p=128))
    wot = wp.tile([128, HD // 128, D], BF16)
    nc.sync.dma_start(out=wot, in_=dwo.rearrange("(b p) d -> p b d", p=128))
    w1t = wp.tile([128, 2, HID], BF16)
    nc.sync.dma_start(out=w1t, in_=dw1.rearrange("(b p) d -> p b d", p=128))
    w2t = wp.tile([128, HID // 128, D], BF16)
    nc.sync.dma_start(out=w2t, in_=dw2.rearrange("(b p) d -> p b d", p=128))
    bqr = wp.tile([128, HD // 128], F32)
    nc.sync.dma_start(out=bqr, in_=dbq)
    bkr = wp.tile([128, HD // 128], F32)
    nc.sync.dma_start(out=bkr, in_=dbk)
    bvb = wp.tile([128, HD], F32)
    nc.sync.dma_start(out=bvb, in_=dbv)
    bor = wp.tile([128, D // 128], F32)
    nc.sync.dma_start(out=bor, in_=dbo)
    b1r = wp.tile([128, HID // 128], F32)
    nc.sync.dma_start(out=b1r, in_=db1)
    b2b = wp.tile([128, D], F32)
    nc.sync.dma_start(out=b2b, in_=db2)
    xqs = hp.tile([128, QB, D], F32)
    nc.sync.dma_start(out=xqs, in_=dxq.rearrange("(b p) d -> p b d", p=128))

    idb = wp.tile([128, 128], BF16)
    make_identity(nc, idb)
    idf = wp.tile([128, 128], F32)
    make_identity(nc, idf)
    ones = wp.tile([128, 128], BF16)
    nc.gpsimd.memset(ones, 1.0)

    # ---- LN1 over full x -> hT [256(2x128), 4096] bf16 ----
    # x loaded in 4 bulk DMAs into fresh tiles: avoids slot-reuse WAR waits
    # (DMACopy instructions only support 2 sync waits).
    xf = []
    for i in range(4):
        xfi = hp.tile([128, 8, D], F32, name=f"xf{i}", tag=f"xf{i}")
        nc.sync.dma_start(
            out=xfi,
            in_=dx[i * 1024:(i + 1) * 1024, :].rearrange("(j p) d -> p j d", p=128))
        xf.append(xfi)
    hT = hp.tile([128, 2, N], BF16)
    for t in range(N // 128):
        ht = work.tile([128, D], BF16, tag="ht")
        _ln_norm(nc, work, xf[t // 8][:, t % 8, :], ht)
        for fc in range(2):
            tp = psp.tile([128, 128], BF16, tag="ps")
            nc.tensor.transpose(tp, ht[:, fc * 128:(fc + 1) * 128], idb)
            nc.vector.tensor_copy(out=hT[:, fc, t * 128:(t + 1) * 128], in_=tp)

    # ---- LN1 over my query block -> hqT [256(2x128), 512] bf16 ----
    hqT = hp.tile([128, 2, NQ], BF16)
    for b in range(QB):
        hqt = work.tile([128, D], BF16, tag="hqt")
        _ln_norm(nc, work, xqs[:, b, :], hqt)
        for fc in range(2):
            tp = psp.tile([128, 128], BF16, tag="ps")
            nc.tensor.transpose(tp, hqt[:, fc * 128:(fc + 1) * 128], idb)
            nc.vector.tensor_copy(out=hqT[:, fc, b * 128:(b + 1) * 128], in_=tp)

    # ---- Q projection for my block: qt [2048(16x128), 512] bf16 ----
    qt = hp.tile([128, HD // 128, NQ], BF16)
    for dc in range(HD // 128):
        qps = psp.tile([128, NQ], F32, tag="ps")
        for ic in range(2):
            nc.tensor.matmul(qps, wqt[:, ic, dc * 128:(dc + 1) * 128],
                             hqT[:, ic, :], start=(ic == 0), stop=(ic == 1))
        nc.scalar.activation(out=qt[:, dc, :], in_=qps, func=Act.Identity,
                             bias=bqr[:, dc:dc + 1], scale=1.0)

    # ---- attention over heads; Wo accumulated in PSUM across heads ----
    ao0 = accp.tile([128, NQ], F32, tag="ao0")
    ao1 = accp.tile([128, NQ], F32, tag="ao1")
    aops = [ao0, ao1]

    def project_head(h):
        """K^T and V for head h -> (kt_h, vt_h) bf16 tiles."""
        kt_h = kvp.tile([128, 2, N], BF16, tag="kt")
        for dc in range(2):
            for ts in range(TS):
                kps = psp.tile([128, 512], F32, tag="ps")
                for ic in range(2):
                    nc.tensor.matmul(
                        kps,
                        wkt[:, ic, (2 * h + dc) * 128:(2 * h + dc + 1) * 128],
                        hT[:, ic, ts * 512:(ts + 1) * 512],
                        start=(ic == 0), stop=(ic == 1))
                nc.scalar.activation(
                    out=kt_h[:, dc, ts * 512:(ts + 1) * 512], in_=kps,
                    func=Act.Identity, bias=bkr[:, 2 * h + dc:2 * h + dc + 1],
                    scale=1.0)
        vt_h = kvp.tile([128, KB, DH], BF16, tag="vt")
        for tb in range(KB):
            vps = psp.tile([128, DH], F32, tag="ps")
            for ic in range(2):
                nc.tensor.matmul(vps, hT[:, ic, tb * 128:(tb + 1) * 128],
                                 wvt[:, ic, h * DH:(h + 1) * DH],
                                 start=(ic == 0), stop=(ic == 1))
            nc.vector.tensor_add(out=vt_h[:, tb, :], in0=vps,
                                 in1=bvb[:, h * DH:(h + 1) * DH])
        return kt_h, vt_h

    kt_h, vt_h = project_head(0)
    for h in range(H):
        au = [accp.tile([128, NQ], F32, tag="au0", name="au0"),
              accp.tile([128, NQ], F32, tag="au1", name="au1")]
        sums = accp.tile([128, NQ], F32, tag="sums")

        # software-pipelined: scores(kb+1) issued before consuming exp(kb)
        sps_tiles = []
        exps_tiles = []

        def scores(kb):
            sps = psp.tile([128, NQ], F32, tag="ps")
            for dc in range(2):
                nc.tensor.matmul(sps, kt_h[:, dc, kb * 128:(kb + 1) * 128],
                                 qt[:, 2 * h + dc, :],
                                 start=(dc == 0), stop=(dc == 1))
            return sps

        def exp_of(sps):
            ex = sp.tile([128, NQ], BF16, tag="expS")
            nc.scalar.activation(out=ex, in_=sps, func=Act.Exp,
                                 scale=float(SCALE))
            return ex

        prev = exp_of(scores(0))
        for kb in range(KB):
            cur = prev
            if kb + 1 < KB:
                prev = exp_of(scores(kb + 1))
            nc.tensor.matmul(sums, ones, cur, start=(kb == 0),
                             stop=(kb == KB - 1), skip_group_check=True)
            for dc in range(2):
                nc.tensor.matmul(au[dc],
                                 vt_h[:, kb, dc * 128:(dc + 1) * 128], cur,
                                 start=(kb == 0), stop=(kb == KB - 1),
                                 skip_group_check=True)

        rs = ap2.tile([128, NQ], F32, tag="rs")
        nc.vector.reciprocal(out=rs, in_=sums)
        attn = [ap2.tile([128, NQ], BF16, tag="attn0", name="attn0"),
                ap2.tile([128, NQ], BF16, tag="attn1", name="attn1")]
        for dc in range(2):
            nc.vector.tensor_mul(out=attn[dc], in0=au[dc], in1=rs)

        # next head's K/V projection keeps PE busy while DVE normalizes
        if h + 1 < H:
            kt_next, vt_next = project_head(h + 1)

        for oc in range(2):
            for dc in range(2):
                nc.tensor.matmul(
                    aops[oc],
                    wot[:, 2 * h + dc, oc * 128:(oc + 1) * 128],
                    attn[dc],
                    start=(h == 0 and dc == 0), stop=(h == H - 1 and dc == 1),
                    skip_group_check=True)
        if h + 1 < H:
            kt_h, vt_h = kt_next, vt_next

    # ---- attn_out^T + bo -> transpose -> +x -> x2 natural ----
    x2 = hp.tile([128, QB, D], F32)
    for oc in range(2):
        aos = work.tile([128, NQ], F32, tag="aos")
        nc.vector.tensor_scalar_add(out=aos, in0=aops[oc],
                                    scalar1=bor[:, oc:oc + 1])
        for qb in range(QB):
            tp = psp.tile([128, 128], F32, tag="ps")
            nc.tensor.transpose(tp, aos[:, qb * 128:(qb + 1) * 128], idf)
            nc.vector.tensor_add(out=x2[:, qb, oc * 128:(oc + 1) * 128],
                                 in0=tp, in1=xqs[:, qb, oc * 128:(oc + 1) * 128])

    # ---- LN2 -> h2T [256(2x128), 512] bf16 ----
    h2T = hp.tile([128, 2, NQ], BF16)
    for b in range(QB):
        h2t = work.tile([128, D], BF16, tag="h2t")
        _ln_norm(nc, work, x2[:, b, :], h2t)
        for fc in range(2):
            tp = psp.tile([128, 128], BF16, tag="ps")
            nc.tensor.transpose(tp, h2t[:, fc * 128:(fc + 1) * 128], idb)
            nc.vector.tensor_copy(out=h2T[:, fc, b * 128:(b + 1) * 128], in_=tp)

    # ---- FFN: z^T = W1 @ h2  (gelu fused with +b1), y natural ----
    zg = hp.tile([128, HID // 128, NQ], BF16)
    for hc in range(HID // 128):
        zps = psp.tile([128, NQ], F32, tag="ps")
        for fc in range(2):
            nc.tensor.matmul(zps, w1t[:, fc, hc * 128:(hc + 1) * 128],
                             h2T[:, fc, :], start=(fc == 0), stop=(fc == 1))
        nc.scalar.activation(out=zg[:, hc, :], in_=zps, func=Act.Gelu,
                             bias=b1r[:, hc:hc + 1], scale=1.0)

    outs = hp.tile([128, QB, D], F32)
    for qb in range(QB):
        yps = psp.tile([128, D], F32, tag="ps")
        for hc in range(HID // 128):
            nc.tensor.matmul(yps, zg[:, hc, qb * 128:(qb + 1) * 128],
                             w2t[:, hc, :], start=(hc == 0),
                             stop=(hc == HID // 128 - 1))
        ytmp = work.tile([128, D], F32, tag="ytmp")
        nc.vector.tensor_add(out=ytmp, in0=yps, in1=b2b)
        nc.vector.tensor_add(out=outs[:, qb, :], in0=ytmp, in1=x2[:, qb, :])
    nc.sync.dma_start(out=dout.rearrange("(b p) d -> p b d", p=128), in_=outs)


_NC = None


def _get_nc():
    global _NC
    if _NC is None:
        _NC = _build()
    return _NC


def _prep_inputs(x, Wq, Wk, Wv, Wo, bo, ln1_g, ln1_b, ln2_g, ln2_b,
                 W1, b1, W2, b2):
    f32 = np.float32
    x = np.ascontiguousarray(x, f32)
    base = {
        "x": x,
        "wqt": np.ascontiguousarray((Wq * ln1_g[None, :]).T.astype(BF)),
        "wkt": np.ascontiguousarray((Wk * ln1_g[None, :]).T.astype(BF)),
        "wvt": np.ascontiguousarray((Wv * ln1_g[None, :]).T.astype(BF)),
        "bqr": np.ascontiguousarray((Wq @ ln1_b).astype(f32).reshape(16, 128).T),
        "bkr": np.ascontiguousarray((Wk @ ln1_b).astype(f32).reshape(16, 128).T),
        "bvb": np.ascontiguousarray(
            np.broadcast_to((Wv @ ln1_b).astype(f32), (128, HD))),
        "wot": np.ascontiguousarray(Wo.T.astype(BF)),
        "bor": np.ascontiguousarray(bo.astype(f32).reshape(2, 128).T),
        "w1t": np.ascontiguousarray((W1 * ln2_g[None, :]).T.astype(BF)),
        "b1r": np.ascontiguousarray(
            (b1 + W1 @ ln2_b).astype(f32).reshape(8, 128).T),
        "w2t": np.ascontiguousarray(W2.T.astype(BF)),
        "b2b": np.ascontiguousarray(np.broadcast_to(b2.astype(f32), (128, D))),
    }
    in_maps = []
    for c in range(NCORES):
        m = dict(base)
        m["xq"] = np.ascontiguousarray(x[c * NQ:(c + 1) * NQ])
        in_maps.append(m)
    return in_maps


def _run(trace=False, **inputs):
    nc = _get_nc()
    in_maps = _prep_inputs(**{k: np.asarray(v) for k, v in inputs.items()})
    res = run_bass_kernel_spmd(nc, in_maps, list(range(NCORES)), trace=trace)
    out = np.concatenate([res.results[c]["out"] for c in range(NCORES)], axis=0)
    return out, res


def kernel(**inputs):
    out, _ = _run(trace=False, **inputs)
    return out


if __name__ == "__main__":
    rng = np.random.default_rng(0)
    ins = {
        "x": rng.standard_normal((N, D), np.float32),
        "Wq": rng.standard_normal((HD, D), np.float32) * 0.02,
        "Wk": rng.standard_normal((HD, D), np.float32) * 0.02,
        "Wv": rng.standard_normal((HD, D), np.float32) * 0.02,
        "Wo": rng.standard_normal((D, HD), np.float32) * 0.02,
        "bo": np.zeros(D, np.float32),
        "ln1_g": np.ones(D, np.float32),
        "ln1_b": np.zeros(D, np.float32),
        "ln2_g": np.ones(D, np.float32),
        "ln2_b": np.zeros(D, np.float32),
        "W1": rng.standard_normal((HID, D), np.float32) * 0.02,
        "b1": np.zeros(HID, np.float32),
        "W2": rng.standard_normal((D, HID), np.float32) * 0.02,
        "b2": np.zeros(D, np.float32),
    }
    out = kernel(**ins)
    print(out.shape, out.dtype, np.abs(out).max())


# revision 21
# speedup vs baseline: 1.1274x; 1.1274x over previous
"""Graphformer encoder layer on 8 trn2 NeuronCores.

Sharding: sequence-parallel over queries. Every core receives the full
input (needed for K/V over all 4096 tokens) plus its own 512-row query
block, and produces 512 rows of the output; the host concatenates.
No collectives.

Layout strategy (per core):
  - LN1 in natural layout, PE-transpose to hT [256(2x128p), 4096].
  - Per head h: K^T [256, 4096] (d on partitions) and V natural
    [4096, 256] projected from hT (bf16).
  - Scores computed TRANSPOSED: S^T[keys, q] = K_h^T.T-chunks @ Q_h^T,
    so softmax sums over keys are partition-dim sums done with a
    ones-matmul on the PE, and probs feed the probs@V matmul without
    any transpose. Softmax normalization is delayed (applied to
    attnU = expS@V before the Wo matmul).
  - Wo output accumulated in PSUM across all 8 heads.
  - Residual/LN2/FFN on the core's own 512 rows.

LayerNorm gains/biases are folded into the (host-transposed) weights:
  Q = ln1norm(x) @ (Wq*g1).T + Wq@b1ln  (same K, V; FFN with ln2).
"""

import os
import sys
import numpy as np

try:
    import concourse.bass as bass
except Exception:  # pragma: no cover
    sys.path.insert(0, "/opt/trn_rl_repo")
    import concourse.bass as bass

import ml_dtypes
import concourse.tile as tile
from concourse import bacc, mybir
from concourse.bass_utils import run_bass_kernel_spmd
from concourse.masks import make_identity

F32 = mybir.dt.float32
F32R = mybir.dt.float32r
BF16 = mybir.dt.bfloat16
BF = ml_dtypes.bfloat16

N = 4096          # tokens
D = 256           # model dim
H = 8             # heads
DH = 256          # head dim  (2 chunks of 128)
HD = H * DH       # 2048
HID = 1024        # ffn hidden
NCORES = 8
NQ = N // NCORES  # 512 queries per core
SCALE = 1.0 / np.sqrt(np.float32(DH))
LN_EPS = 1e-5

KB = N // 128     # 32 key blocks of 128
QB = NQ // 128    # 4 query blocks of 128
TS = N // 512     # 8 token slabs of 512 (matmul moving max for fp32)


def _r(ap):
    """Bitcast fp32 -> float32r: full-rate (1 cyc/row) PE streaming."""
    return ap.bitcast(F32R)


def _ln_norm(nc, pool, xt, out):
    """out = (xt - mean(xt)) * rsqrt(var(xt)+eps), stats on DVE, sqrt on ACT."""
    stats = pool.tile([128, 6], F32, tag="ln_stats")
    nc.vector.bn_stats(out=stats, in_=xt)
    mv = pool.tile([128, 2], F32, tag="ln_mv")
    nc.vector.bn_aggr(out=mv, in_=stats)
    vpe = pool.tile([128, 1], F32, tag="ln_vpe")
    nc.vector.tensor_scalar_add(out=vpe, in0=mv[:, 1:2], scalar1=LN_EPS)
    std = pool.tile([128, 1], F32, tag="ln_std")
    nc.scalar.sqrt(std, vpe)
    rsig = pool.tile([128, 1], F32, tag="ln_rsig")
    nc.vector.reciprocal(out=rsig, in_=std)
    nc.vector.tensor_scalar(out=out, in0=xt, scalar1=mv[:, 0:1], scalar2=rsig,
                            op0=mybir.AluOpType.subtract,
                            op1=mybir.AluOpType.mult)


def _build():
    nc = bacc.Bacc("TRN2", target_bir_lowering=False, debug=False)

    dx = nc.dram_tensor("x", [N, D], F32, kind="ExternalInput").ap()
    dxq = nc.dram_tensor("xq", [NQ, D], F32, kind="ExternalInput").ap()
    dwq = nc.dram_tensor("wqt", [D, HD], BF16, kind="ExternalInput").ap()
    dwk = nc.dram_tensor("wkt", [D, HD], BF16, kind="ExternalInput").ap()
    dwv = nc.dram_tensor("wvt", [D, HD], BF16, kind="ExternalInput").ap()
    dbq = nc.dram_tensor("bqr", [128, HD // 128], F32, kind="ExternalInput").ap()
    dbk = nc.dram_tensor("bkr", [128, HD // 128], F32, kind="ExternalInput").ap()
    dbv = nc.dram_tensor("bvb", [128, HD], F32, kind="ExternalInput").ap()
    dwo = nc.dram_tensor("wot", [HD, D], BF16, kind="ExternalInput").ap()
    dbo = nc.dram_tensor("bor", [128, D // 128], F32, kind="ExternalInput").ap()
    dw1 = nc.dram_tensor("w1t", [D, HID], BF16, kind="ExternalInput").ap()
    db1 = nc.dram_tensor("b1r", [128, HID // 128], F32, kind="ExternalInput").ap()
    dw2 = nc.dram_tensor("w2t", [HID, D], BF16, kind="ExternalInput").ap()
    db2 = nc.dram_tensor("b2b", [128, D], F32, kind="ExternalInput").ap()
    dout = nc.dram_tensor("out", [NQ, D], F32, kind="ExternalOutput").ap()

    with tile.TileContext(nc) as tc:
        import contextlib
        with contextlib.ExitStack() as ctx:
            _body(ctx, tc, nc, dx, dxq, dwq, dwk, dwv, dbq, dbk, dbv,
                  dwo, dbo, dw1, db1, dw2, db2, dout)
    nc.compile()
    return nc


def _body(ctx, tc, nc, dx, dxq, dwq, dwk, dwv, dbq, dbk, dbv,
          dwo, dbo, dw1, db1, dw2, db2, dout):
    Act = mybir.ActivationFunctionType

    wp = ctx.enter_context(tc.tile_pool(name="weights", bufs=1))
    hp = ctx.enter_context(tc.tile_pool(name="persist", bufs=1))
    kvp = ctx.enter_context(tc.tile_pool(name="kv", bufs=1))
    work = ctx.enter_context(tc.tile_pool(name="work", bufs=3))
    sp = ctx.enter_context(tc.tile_pool(name="probs", bufs=3))
    ap2 = ctx.enter_context(tc.tile_pool(name="attn", bufs=2))
    accp = ctx.enter_context(tc.tile_pool(name="acc", bufs=1, space="PSUM"))
    psp = ctx.enter_context(tc.tile_pool(name="pscr", bufs=2, space="PSUM"))

    # ---- weights / constants ----
    wqt = wp.tile([128, 2, HD], BF16)
    nc.sync.dma_start(out=wqt, in_=dwq.rearrange("(b p) d -> p b d", p=128))
    wkt = wp.tile([128, 2, HD], BF16)
    nc.sync.dma_start(out=wkt, in_=dwk.rearrange("(b p) d -> p b d", p=128))
    wvt = wp.tile([128, 2, HD], BF16)
    nc.sync.dma_start(out=wvt, in_=dwv.rearrange("(b p) d -> p b d", p=128))
    wot = wp.tile([128, HD // 128, D], BF16)
    nc.sync.dma_start(out=wot, in_=dwo.rearrange("(b p) d -> p b d", p=128))
    w1t = wp.tile([128, 2, HID], BF16)
    nc.sync.dma_start(out=w1t, in_=dw1.rearrange("(b p) d -> p b d", p=128))
    w2t = wp.tile([128, HID // 128, D], BF16)
    nc.sync.dma_start(out=w2t, in_=dw2.rearrange("(b p) d -> p b d", p=128))
    bqr = wp.tile([128, HD // 128], F32)
    nc.sync.dma_start(out=bqr, in_=dbq)
    bkr = wp.tile([128, HD // 128], F32)
    nc.sync.dma_start(out=bkr, in_=dbk)
    bvb = wp.tile([128, HD], F32)
    nc.sync.dma_start(out=bvb, in_=dbv)
    bor = wp.tile([128, D // 128], F32)
    nc.sync.dma_start(out=bor, in_=dbo)
    b1r = wp.tile([128, HID // 128], F32)
    nc.sync.dma_start(out=b1r, in_=db1)
    b2b = wp.tile([128, D], F32)
    nc.sync.dma_start(out=b2b, in_=db2)
    xqs = hp.tile([128, QB, D], F32)
    nc.sync.dma_start(out=xqs, in_=dxq.rearrange("(b p) d -> p b d", p=128))

    idb = wp.tile([128, 128], BF16)
    make_identity(nc, idb)
    idf = wp.tile([128, 128], F32)
    make_identity(nc, idf)
    onesf = wp.tile([128, 128], F32)
    nc.gpsimd.memset(onesf, 1.0)

    # ---- LN1 over full x -> hT [256(2x128), 4096] bf16 ----
    # x loaded in 4 bulk DMAs into fresh tiles: avoids slot-reuse WAR waits
    # (DMACopy instructions only support 2 sync waits).
    xf = []
    for i in range(4):
        xfi = hp.tile([128, 8, D], F32, name=f"xf{i}", tag=f"xf{i}")
        nc.sync.dma_start(
            out=xfi,
            in_=dx[i * 1024:(i + 1) * 1024, :].rearrange("(j p) d -> p j d", p=128))
        xf.append(xfi)
    hT = hp.tile([128, 2, N], BF16)
    for t in range(N // 128):
        ht = work.tile([128, D], BF16, tag="ht")
        _ln_norm(nc, work, xf[t // 8][:, t % 8, :], ht)
        for fc in range(2):
            tp = psp.tile([128, 128], BF16, tag="ps")
            nc.tensor.transpose(tp, ht[:, fc * 128:(fc + 1) * 128], idb)
            nc.vector.tensor_copy(out=hT[:, fc, t * 128:(t + 1) * 128], in_=tp)

    # ---- LN1 over my query block -> hqT [256(2x128), 512] bf16 ----
    hqT = hp.tile([128, 2, NQ], BF16)
    for b in range(QB):
        hqt = work.tile([128, D], BF16, tag="hqt")
        _ln_norm(nc, work, xqs[:, b, :], hqt)
        for fc in range(2):
            tp = psp.tile([128, 128], BF16, tag="ps")
            nc.tensor.transpose(tp, hqt[:, fc * 128:(fc + 1) * 128], idb)
            nc.vector.tensor_copy(out=hqT[:, fc, b * 128:(b + 1) * 128], in_=tp)

    # ---- Q projection for my block: qt [2048(16x128), 512] bf16 ----
    qt = hp.tile([128, HD // 128, NQ], BF16)
    for dc in range(HD // 128):
        qps = psp.tile([128, NQ], F32, tag="ps")
        for ic in range(2):
            nc.tensor.matmul(qps, wqt[:, ic, dc * 128:(dc + 1) * 128],
                             hqT[:, ic, :], start=(ic == 0), stop=(ic == 1))
        nc.scalar.activation(out=qt[:, dc, :], in_=qps, func=Act.Identity,
                             bias=bqr[:, dc:dc + 1], scale=1.0)

    # ---- attention over heads; Wo accumulated in PSUM across heads ----
    ao0 = accp.tile([128, NQ], F32, tag="ao0")
    ao1 = accp.tile([128, NQ], F32, tag="ao1")
    aops = [ao0, ao1]

    def project_head(h):
        """K^T and V for head h -> (kt_h, vt_h) bf16 tiles."""
        kt_h = kvp.tile([128, 2, N], BF16, tag="kt")
        for dc in range(2):
            for ts in range(TS):
                kps = psp.tile([128, 512], F32, tag="ps")
                for ic in range(2):
                    nc.tensor.matmul(
                        kps,
                        wkt[:, ic, (2 * h + dc) * 128:(2 * h + dc + 1) * 128],
                        hT[:, ic, ts * 512:(ts + 1) * 512],
                        start=(ic == 0), stop=(ic == 1))
                nc.scalar.activation(
                    out=kt_h[:, dc, ts * 512:(ts + 1) * 512], in_=kps,
                    func=Act.Identity, bias=bkr[:, 2 * h + dc:2 * h + dc + 1],
                    scale=1.0)
        vt_h = kvp.tile([128, KB, DH], BF16, tag="vt")
        for tb in range(KB):
            vps = psp.tile([128, DH], F32, tag="ps")
            for ic in range(2):
                nc.tensor.matmul(vps, hT[:, ic, tb * 128:(tb + 1) * 128],
                                 wvt[:, ic, h * DH:(h + 1) * DH],
                                 start=(ic == 0), stop=(ic == 1))
            nc.vector.tensor_add(out=vt_h[:, tb, :], in0=vps,
                                 in1=bvb[:, h * DH:(h + 1) * DH])
        return kt_h, vt_h

    kt_h, vt_h = project_head(0)
    for h in range(H):
        au = [accp.tile([128, NQ], F32, tag="au0", name="au0"),
              accp.tile([128, NQ], F32, tag="au1", name="au1")]
        # per-lane partial softmax sums accumulated on DVE (saves PE cycles);
        # cross-partition total via one fp32 ones-matmul per head below
        ssum = ap2.tile([128, NQ], F32, tag="ssum")

        # software-pipelined: scores(kb+1) issued before consuming exp(kb)
        sps_tiles = []
        exps_tiles = []

        def scores(kb):
            sps = psp.tile([128, NQ], F32, tag="ps")
            for dc in range(2):
                nc.tensor.matmul(sps, kt_h[:, dc, kb * 128:(kb + 1) * 128],
                                 qt[:, 2 * h + dc, :],
                                 start=(dc == 0), stop=(dc == 1))
            return sps

        def exp_of(sps):
            ex = sp.tile([128, NQ], BF16, tag="expS")
            nc.scalar.activation(out=ex, in_=sps, func=Act.Exp,
                                 scale=float(SCALE))
            return ex

        prev = exp_of(scores(0))
        for kb in range(KB):
            cur = prev
            if kb + 1 < KB:
                prev = exp_of(scores(kb + 1))
            if kb == 0:
                nc.vector.tensor_copy(out=ssum, in_=cur)
            else:
                nc.vector.tensor_add(out=ssum, in0=ssum, in1=cur)
            for dc in range(2):
                nc.tensor.matmul(au[dc],
                                 vt_h[:, kb, dc * 128:(dc + 1) * 128], cur,
                                 start=(kb == 0), stop=(kb == KB - 1),
                                 skip_group_check=True)

        sums = accp.tile([128, NQ], F32, tag="sums")
        nc.tensor.matmul(sums, onesf, ssum, start=True, stop=True)
        rs = ap2.tile([128, NQ], F32, tag="rs")
        nc.vector.reciprocal(out=rs, in_=sums)
        attn = [ap2.tile([128, NQ], BF16, tag="attn0", name="attn0"),
                ap2.tile([128, NQ], BF16, tag="attn1", name="attn1")]
        for dc in range(2):
            nc.vector.tensor_mul(out=attn[dc], in0=au[dc], in1=rs)

        # next head's K/V projection keeps PE busy while DVE normalizes
        if h + 1 < H:
            kt_next, vt_next = project_head(h + 1)

        for oc in range(2):
            for dc in range(2):
                nc.tensor.matmul(
                    aops[oc],
                    wot[:, 2 * h + dc, oc * 128:(oc + 1) * 128],
                    attn[dc],
                    start=(h == 0 and dc == 0), stop=(h == H - 1 and dc == 1),
                    skip_group_check=True)
        if h + 1 < H:
            kt_h, vt_h = kt_next, vt_next

    # ---- attn_out^T + bo -> transpose -> +x -> x2 natural ----
    x2 = hp.tile([128, QB, D], F32)
    for oc in range(2):
        aos = work.tile([128, NQ], F32, tag="aos")
        nc.vector.tensor_scalar_add(out=aos, in0=aops[oc],
                                    scalar1=bor[:, oc:oc + 1])
        for qb in range(QB):
            tp = psp.tile([128, 128], F32, tag="ps")
            nc.tensor.transpose(tp, aos[:, qb * 128:(qb + 1) * 128], idf)
            nc.vector.tensor_add(out=x2[:, qb, oc * 128:(oc + 1) * 128],
                                 in0=tp, in1=xqs[:, qb, oc * 128:(oc + 1) * 128])

    # ---- LN2 -> h2T [256(2x128), 512] bf16 ----
    h2T = hp.tile([128, 2, NQ], BF16)
    for b in range(QB):
        h2t = work.tile([128, D], BF16, tag="h2t")
        _ln_norm(nc, work, x2[:, b, :], h2t)
        for fc in range(2):
            tp = psp.tile([128, 128], BF16, tag="ps")
            nc.tensor.transpose(tp, h2t[:, fc * 128:(fc + 1) * 128], idb)
            nc.vector.tensor_copy(out=h2T[:, fc, b * 128:(b + 1) * 128], in_=tp)

    # ---- FFN: z^T = W1 @ h2  (gelu fused with +b1), y natural ----
    zg = hp.tile([128, HID // 128, NQ], BF16)
    for hc in range(HID // 128):
        zps = psp.tile([128, NQ], F32, tag="ps")
        for fc in range(2):
            nc.tensor.matmul(zps, w1t[:, fc, hc * 128:(hc + 1) * 128],
                             h2T[:, fc, :], start=(fc == 0), stop=(fc == 1))
        nc.scalar.activation(out=zg[:, hc, :], in_=zps, func=Act.Gelu,
                             bias=b1r[:, hc:hc + 1], scale=1.0)

    outs = hp.tile([128, QB, D], F32)
    for qb in range(QB):
        yps = psp.tile([128, D], F32, tag="ps")
        for hc in range(HID // 128):
            nc.tensor.matmul(yps, zg[:, hc, qb * 128:(qb + 1) * 128],
                             w2t[:, hc, :], start=(hc == 0),
                             stop=(hc == HID // 128 - 1))
        ytmp = work.tile([128, D], F32, tag="ytmp")
        nc.vector.tensor_add(out=ytmp, in0=yps, in1=b2b)
        nc.vector.tensor_add(out=outs[:, qb, :], in0=ytmp, in1=x2[:, qb, :])
    nc.sync.dma_start(out=dout.rearrange("(b p) d -> p b d", p=128), in_=outs)


_NC = None


def _get_nc():
    global _NC
    if _NC is None:
        _NC = _build()
    return _NC


def _prep_inputs(x, Wq, Wk, Wv, Wo, bo, ln1_g, ln1_b, ln2_g, ln2_b,
                 W1, b1, W2, b2):
    f32 = np.float32
    x = np.ascontiguousarray(x, f32)
    base = {
        "x": x,
        "wqt": np.ascontiguousarray((Wq * ln1_g[None, :]).T.astype(BF)),
        "wkt": np.ascontiguousarray((Wk * ln1_g[None, :]).T.astype(BF)),
        "wvt": np.ascontiguousarray((Wv * ln1_g[None, :]).T.astype(BF)),
        "bqr": np.ascontiguousarray((Wq @ ln1_b).astype(f32).reshape(16, 128).T),
        "bkr": np.ascontiguousarray((Wk @ ln1_b).astype(f32).reshape(16, 128).T),
        "bvb": np.ascontiguousarray(
            np.broadcast_to((Wv @ ln1_b).astype(f32), (128, HD))),
        "wot": np.ascontiguousarray(Wo.T.astype(BF)),
        "bor": np.ascontiguousarray(bo.astype(f32).reshape(2, 128).T),
        "w1t": np.ascontiguousarray((W1 * ln2_g[None, :]).T.astype(BF)),
        "b1r": np.ascontiguousarray(
            (b1 + W1 @ ln2_b).astype(f32).reshape(8, 128).T),
        "w2t": np.ascontiguousarray(W2.T.astype(BF)),
        "b2b": np.ascontiguousarray(np.broadcast_to(b2.astype(f32), (128, D))),
    }
    in_maps = []
    for c in range(NCORES):
        m = dict(base)
        m["xq"] = np.ascontiguousarray(x[c * NQ:(c + 1) * NQ])
        in_maps.append(m)
    return in_maps


def _run(trace=False, **inputs):
    nc = _get_nc()
    in_maps = _prep_inputs(**{k: np.asarray(v) for k, v in inputs.items()})
    res = run_bass_kernel_spmd(nc, in_maps, list(range(NCORES)), trace=trace)
    out = np.concatenate([res.results[c]["out"] for c in range(NCORES)], axis=0)
    return out, res


def kernel(**inputs):
    out, _ = _run(trace=False, **inputs)
    return out


if __name__ == "__main__":
    rng = np.random.default_rng(0)
    ins = {
        "x": rng.standard_normal((N, D), np.float32),
        "Wq": rng.standard_normal((HD, D), np.float32) * 0.02,
        "Wk": rng.standard_normal((HD, D), np.float32) * 0.02,
        "Wv": rng.standard_normal((HD, D), np.float32) * 0.02,
        "Wo": rng.standard_normal((D, HD), np.float32) * 0.02,
        "bo": np.zeros(D, np.float32),
        "ln1_g": np.ones(D, np.float32),
        "ln1_b": np.zeros(D, np.float32),
        "ln2_g": np.ones(D, np.float32),
        "ln2_b": np.zeros(D, np.float32),
        "W1": rng.standard_normal((HID, D), np.float32) * 0.02,
        "b1": np.zeros(HID, np.float32),
        "W2": rng.standard_normal((D, HID), np.float32) * 0.02,
        "b2": np.zeros(D, np.float32),
    }
    out = kernel(**ins)
    print(out.shape, out.dtype, np.abs(out).max())
